# revision 38
# baseline (speedup 1.0000x reference)
"""GAT regressor (2x GATConv + mean-pool + MLP) on 8 Trainium2 cores.

Strategy (dst-sharded, single fused launch, renumbered tables):
- Edges sorted by destination; core c owns dst nodes [c*6250, (c+1)*6250).
- Within a core, nodes are renumbered by descending in-degree so the padded
  CSR (one [128 nodes x K_t slots] tile per 128 nodes) wastes ~7% slots.
- All gather tables are laid out in the RENUMBERED row space (NC*NLP+1 rows,
  last row is the padding dummy), so ONE index array (idx2) serves both GAT
  layers: layer 1 gathers 80B records [x(16), a_s1(4)] from T1, layer 2
  gathers 144B records [h2(32), a_s2(1), pad(3)] from T2.
- Single SPMD launch on 8 cores with on-device collectives:
    AllGather of the weight-pack shards and of the (u8-quantized) x shards;
    AllGather of the per-core T2 parts -> full T2 on every core;
    AllReduce of the pooled [G, C2] partials -> replicated MLP head.
- Self-loop slots are implicit (device fills slot 0 of each row from rsel),
  so only the raw edges ship.
- All per-core host->device traffic travels as ONE u16 buffer of ~0.52 MB:
  [wpk shard (f32), x (u8, scale in wpk), idx2 (u16), rsel (u16), gid (u8)].
  The compiled executable is cached so repeat calls pay only transfer +
  execution; the axon tunnel (~12 ms/MB, serialized across cores) dominates.
"""
import numpy as np

import concourse.bass as bass
import concourse.tile as ctile
from concourse import mybir, bass2jax
from concourse.vector_clock import ScopedClock
from concourse.masks import make_identity

F32 = mybir.dt.float32
F16 = mybir.dt.float16
I32 = mybir.dt.int32
U16 = mybir.dt.uint16
U8 = mybir.dt.uint8
AX = mybir.AxisListType
OP = mybir.AluOpType
ACT = mybir.ActivationFunctionType

N = 50000
E0 = 1_600_000
G = 100
IN = 16
H1, C1 = 4, 32
F1 = H1 * C1              # 128
C2 = 32
NEG = 0.2
NC = 8
NL = N // NC              # 6250
P = 128
NT = (NL + P - 1) // P    # 49
NLP = NT * P              # 6272 rows per core (renumbered, padded)
NROWS = NC * NLP          # 50176
TROWS = NROWS + 1         # + dummy row
NLQ = NROWS // 8          # 6272 phase-A columns
REC1 = 20                 # [x(16), a_s1(4)]
REC2 = 36                 # [h2(32), a_s2(1), pad(3)]
GNT = NT + 1              # gid u8 columns padded even (50)
RG = [list(range(NC))]


# ---------------------------------------------------------------------------
# TileContext tail-drain patch: this walrus build allows only one sem wait per
# CTRL instruction; spread the kernel-tail drain waits over several drains.
def _patched_drain_and_barrier(self, tick_clock, wait_clock):
    drain_inst = self.nc.sync.drain()
    extras = [self.nc.sync.drain() for _ in range(40)]
    wait_clock.add_sem_waits(
        drain_inst.ins, ScopedClock({None: tick_clock.global_clock})
    )
    si = drain_inst.ins.sync_info
    waits = list(si.on_wait or []) if si is not None else []
    if len(waits) > 1:
        si.on_wait = waits[:1]
        for i, w in enumerate(waits[1:]):
            esi = extras[i].ins.sync_info
            if esi is None:
                extras[i].ins.sync_info = mybir.SyncInfo(on_wait=[w], on_update=[])
            else:
                esi.on_wait = [w]
    self.nc.all_engine_barrier()
    popped = self.nc._tile_sem_poison_stack.pop()
    assert popped is self._sem_poison
    self.nc.clear_and_free_semaphores(list(self.sems.allocated().values()))
    self.nc.all_engine_barrier()


ctile.TileContext._drain_and_barrier = _patched_drain_and_barrier


def fix_multiwait(nc):
    """This walrus build allows only one sem wait per instruction: hoist all
    but one wait of any instruction onto same-engine NOPs inserted before it."""
    for f in nc.m.functions:
        for bb in f.blocks:
            lst = bb.instructions
            i = 0
            while i < len(lst):
                inst = lst[i]
                si = inst.sync_info
                waits = list(si.on_wait) if si and si.on_wait else []
                if len(waits) > 1:
                    si.on_wait = waits[-1:]
                    for w in waits[:-1]:
                        nop = mybir.InstNoOp(
                            name=nc.get_next_instruction_name(), ins=[], outs=[])
                        nop.engine = inst.engine
                        nop.sync_info = mybir.SyncInfo(on_wait=[w], on_update=[])
                        nc.register_instruction(nop)
                        lst.insert(i, nop)
                        i += 1
                i += 1


def vap(t, off, dims):
    """Flat (DRAM) AP view with extra element offset and [step,count] dims."""
    a = t[:] if not isinstance(t, bass.AP) else t
    return bass.AP(tensor=a.tensor, offset=a.offset + off, ap=dims)


def svap(t, off, free_dims):
    """SBUF AP view: keeps the base AP's partition pair (partition step must
    stay the tile's free pitch), custom free [step,count] dims + elem offset."""
    a = t[:] if not isinstance(t, bass.AP) else t
    return bass.AP(tensor=a.tensor, offset=a.offset + off,
                   ap=[list(a.ap[0])] + free_dims)


# ---------------------------------------------------------------------------
# host preprocessing: pure index/layout work
def _ranges(d):
    """concat([arange(d0), arange(d1), ...]) for int array d."""
    tot = int(d.sum())
    if tot == 0:
        return np.zeros(0, np.int64)
    csum = np.zeros(len(d), np.int64)
    np.cumsum(d[:-1], out=csum[1:])
    return np.arange(tot, dtype=np.int64) - np.repeat(csum, d)


def host_prep(x, edge_index, batch):
    x = np.asarray(x, np.float32)
    ei = np.asarray(edge_index).astype(np.int64)
    batch = np.asarray(batch).astype(np.int64)

    # CSR over the raw edges only; the self-loop every row gets is implicit
    # (the device fills slot 0 of each row with the row's own id).
    src = ei[0].astype(np.int32)
    dst = ei[1].astype(np.int32)
    order = np.argsort(dst, kind="stable")
    src_s = src[order]
    dst_s = dst[order]
    deg = np.bincount(dst_s, minlength=N)
    rowptr = np.zeros(N + 1, np.int64)
    np.cumsum(deg, out=rowptr[1:])

    perms, deg_sorted_all = [], []
    for c in range(NC):
        lo = c * NL
        d_local = deg[lo:lo + NL]
        perm = np.argsort(-d_local, kind="stable").astype(np.int64)
        perms.append(perm)
        deg_sorted_all.append(d_local[perm])

    # global per-tile K schedule (shared program across cores); K counts the
    # implicit self-loop slot, so K-1 edge slots are shipped per row.
    Ks = []
    for t in range(NT):
        k = 0
        for c in range(NC):
            seg = deg_sorted_all[c][t * P:(t + 1) * P]
            if len(seg):
                k = max(k, int(seg.max()) + 1)
        Ks.append(max(4, k))
    L1TOT = P * sum(K - 1 for K in Ks)

    # renumber map: orig node -> global renumbered row
    t2row = np.empty(N, np.int32)
    for c in range(NC):
        lo = c * NL
        inv = np.empty(NL, np.int64)
        inv[perms[c]] = np.arange(NL, dtype=np.int64)
        t2row[lo:lo + NL] = (c * NLP + inv).astype(np.int32)

    xscale = np.float32(max(np.abs(x).max(), 1e-30) / 127.0)

    idx2s, xps, rsels, gidfs = [], [], [], []
    for c in range(NC):
        lo = c * NL
        perm = perms[c]
        dsort = deg_sorted_all[c]
        idx2 = np.full(L1TOT, NROWS, np.uint16)
        off = 0
        for t in range(NT):
            KS = Ks[t] - 1
            l0, l1 = t * P, min(t * P + P, NL)
            nrow = l1 - l0
            nodes = lo + perm[l0:l1]
            d = dsort[l0:l1].astype(np.int64)
            tbl = np.full((P, KS), NROWS, np.uint16)
            take = rowptr[nodes].repeat(d) + _ranges(d)
            mask = np.arange(KS)[None, :] < d[:, None]
            tbl[:nrow][mask] = t2row[src_s[take]].astype(np.uint16)
            idx2[off:off + P * KS] = tbl.ravel()
            off += P * KS
        idx2s.append(idx2)

        xp = np.zeros((NLP, IN), np.uint8)
        xp[:NL] = np.clip(np.rint(x[lo + perm] / xscale) + 128, 1, 255
                          ).astype(np.uint8)
        xp[NL:] = 128
        xps.append(xp)

        rsel = (c * NLP + np.arange(NT, dtype=np.int64)[None, :] * P
                + np.arange(P, dtype=np.int64)[:, None]).astype(np.uint16)
        rsels.append(np.ascontiguousarray(rsel))

        g_of_l = np.full(NLP, 255, np.uint8)
        g_of_l[:NL] = batch[lo + perm].astype(np.uint8)
        gid = np.full((P, GNT), 255, np.uint8)
        gid[:, :NT] = g_of_l.reshape(NT, P).T
        gidfs.append(gid)

    cnt = np.bincount(batch, minlength=G).astype(np.float32)

    return dict(Ks=Ks, L1TOT=L1TOT, idx2s=idx2s, xps=xps, rsels=rsels,
                gidfs=gidfs, cnt=cnt, xscale=xscale)


# wpack layout (flat f32 offsets)
W_A8W = 0                       # [128, 64]
W_W1B = W_A8W + 128 * 64        # [64, 128]
W_B1 = W_W1B + 64 * 128         # [128]
W_W2 = W_B1 + 128               # [128, 32]
W_AT2 = W_W2 + 128 * 32         # [32, 2]
W_B2 = W_AT2 + 64               # [32]
W_WH1 = W_B2 + C2               # [32, 64]
W_BH1 = W_WH1 + 32 * 64         # [64]
W_WH2 = W_BH1 + 64              # [64]
W_BH2 = W_WH2 + 64              # [1]
W_CNT = W_BH2 + 1               # [100]
W_IOT = W_CNT + G               # [100]
W_XSC = W_IOT + G               # [1] x dequant scale
WPK = W_XSC + 1


def fold_weights(W1, att_src1, att_dst1, b1, W2, att_src2, att_dst2, b2,
                 Wh1, bh1, Wh2, bh2, cnt, xscale):
    W1 = np.asarray(W1, np.float32)
    W1r = W1.reshape(IN, H1, C1)
    Vs = np.einsum("fhc,hc->fh", W1r, np.asarray(att_src1, np.float32))
    Vd = np.einsum("fhc,hc->fh", W1r, np.asarray(att_dst1, np.float32))
    # A8 row layout: rows 0:32 = a_s (g*4+h), rows 32:64 = a_d (g*4+h) so that
    # DMA reads start at partition 0 / 32 (quadrant rule).
    A8_lhsT = np.zeros((P, 64), np.float32)
    for g in range(NC):
        A8_lhsT[g * IN:(g + 1) * IN, g * 4:(g + 1) * 4] = Vs
        A8_lhsT[g * IN:(g + 1) * IN, 32 + g * 4:32 + (g + 1) * 4] = Vd
    W1blk = np.zeros((64, F1), np.float32)
    for h in range(H1):
        W1blk[h * IN:(h + 1) * IN, h * C1:(h + 1) * C1] = W1r[:, h, :]
    att2 = np.stack([np.asarray(att_src2, np.float32).ravel(),
                     np.asarray(att_dst2, np.float32).ravel()], 1)  # [32, 2]

    w = np.zeros(WPK, np.float32)
    w[W_A8W:W_A8W + 128 * 64] = A8_lhsT.ravel()
    w[W_W1B:W_W1B + 64 * 128] = W1blk.ravel()
    w[W_B1:W_B1 + 128] = np.asarray(b1, np.float32).ravel()
    w[W_W2:W_W2 + 128 * 32] = np.asarray(W2, np.float32).ravel()
    w[W_AT2:W_AT2 + 64] = att2.ravel()
    w[W_B2:W_B2 + C2] = np.asarray(b2, np.float32).ravel()
    w[W_WH1:W_WH1 + 32 * 64] = np.asarray(Wh1, np.float32).ravel()
    w[W_BH1:W_BH1 + 64] = np.asarray(bh1, np.float32).ravel()
    w[W_WH2:W_WH2 + 64] = np.asarray(Wh2, np.float32).ravel()
    w[W_BH2] = np.float32(np.asarray(bh2).ravel()[0])
    w[W_CNT:W_CNT + G] = cnt
    w[W_IOT:W_IOT + G] = np.arange(G, dtype=np.float32)
    w[W_XSC] = xscale
    return w


# single per-core input pack (u16 elements): [wpk shard (f32), xp (u8),
# idx2 (u16), rsel (u16), gid (u8)]
SH32 = -(-WPK // NC)            # f32 elems of each core's wpk shard
W16 = 2 * SH32
XPO = W16
IXO = XPO + NLP * IN // 2


def _pack_layout(L1TOT):
    RSO = IXO + L1TOT
    GIO = RSO + P * NT
    TOT = GIO + P * GNT // 2
    return RSO, GIO, TOT


def _build_packs(prep, wpack):
    RSO, GIO, TOT = _pack_layout(prep["L1TOT"])
    wsh = np.zeros(SH32 * NC, np.float32)
    wsh[:WPK] = wpack
    packs = []
    for c in range(NC):
        pk = np.empty(TOT, np.uint16)
        pk[0:W16] = wsh[c * SH32:(c + 1) * SH32].view(np.uint16)
        pk[XPO:IXO] = prep["xps"][c].ravel().view(np.uint16)
        pk[IXO:RSO] = prep["idx2s"][c]
        pk[RSO:GIO] = prep["rsels"][c].ravel()
        pk[GIO:TOT] = prep["gidfs"][c].ravel().view(np.uint16)
        packs.append(pk)
    return packs


# ---------------------------------------------------------------------------
def edge_softmax_aggregate(nc, tc, pools, idx_dram, tbl_dram, a_d_view, t, K,
                           rec, nmsg, nheads, self_col, out_cb):
    """Per-tile padded-CSR gather + segment softmax + weighted aggregation.

    a_d_view: AP [128, nheads] (per-dst attention term, this tile)
    rec: record width; nmsg: message feature count (cols 0:nmsg of record);
    a_s lives at record col nmsg..nmsg+nheads-1.
    self_col: AP [128, 1] i32, each row's own table index (implicit self-loop
    slot 0; idx_dram supplies the other K-1 slots).
    out_cb(OPS): callback receiving [128, nheads*nmsg] aggregated+normalized.
    """
    work, psum = pools["work"], pools["psum"]
    H = nheads
    it16 = work.tile([P, K - 1], U16, tag="it16")
    nc.sync.dma_start(out=it16[:], in_=idx_dram)
    it = work.tile([P, K], I32, tag="it")
    nc.vector.tensor_copy(out=it[:, 0:1], in_=self_col)
    nc.vector.tensor_copy(out=it[:, 1:K], in_=it16[:])
    g_ = work.tile([P, K * rec], F32, tag="g")
    # HW indirect DMA consumes ONE offset per partition (per contiguous dest
    # run), so gather one k-slot (128 rows) per instruction.
    for k in range(K):
        nc.gpsimd.indirect_dma_start(
            out=g_[:, k * rec:(k + 1) * rec], out_offset=None, in_=tbl_dram,
            in_offset=bass.IndirectOffsetOnAxis(ap=it[:, k:k + 1], axis=0))

    # logits L0[p, h, k] = a_s[src] + a_d[dst]
    L0 = work.tile([P, H * K], F32, tag="L0")
    nc.vector.tensor_tensor(
        out=L0[:],
        in0=svap(g_, nmsg, [[1, H], [rec, K]]),
        in1=svap(a_d_view, 0, [[1, H], [0, K]]),
        op=OP.add)
    # leaky relu
    Lm = work.tile([P, H * K], F32, tag="Lm")
    nc.vector.tensor_scalar_mul(Lm[:], L0[:], NEG)
    nc.vector.tensor_tensor(out=Lm[:], in0=L0[:], in1=Lm[:], op=OP.max)
    # segment max / exp / denom
    m = work.tile([P, H], F32, tag="m")
    nc.vector.tensor_reduce(
        out=m[:], in_=svap(Lm, 0, [[K, H], [1, K]]),
        axis=AX.X, op=OP.max)
    S = work.tile([P, H * K], F32, tag="S")
    nc.vector.tensor_tensor(
        out=S[:], in0=Lm[:],
        in1=svap(m, 0, [[1, H], [0, K]]), op=OP.subtract)
    # clamp: pad slots carry ~-2e29 logits; HW ACT Exp tables need sane range
    nc.vector.tensor_scalar_max(S[:], S[:], -80.0)
    EX = work.tile([P, H * K], F32, tag="EX")
    nc.scalar.activation(EX[:], S[:], ACT.Exp)
    den = work.tile([P, H], F32, tag="den")
    nc.vector.tensor_reduce(
        out=den[:], in_=svap(EX, 0, [[K, H], [1, K]]),
        axis=AX.X, op=OP.add)
    dr = work.tile([P, H], F32, tag="dr")
    nc.vector.tensor_scalar_add(dr[:], den[:], 1e-16)
    nc.vector.reciprocal(dr[:], dr[:])
    # weighted aggregation: OP[p,h,f] = sum_k EX[p,h,k] * msg[p,k,f]
    prod = work.tile([P, H * K * nmsg], F32, tag="prod")
    nc.vector.tensor_tensor(
        out=prod[:],
        in0=svap(EX, 0, [[K, H], [1, K], [0, nmsg]]),
        in1=svap(g_, 0, [[0, H], [rec, K], [1, nmsg]]),
        op=OP.mult)
    agg = work.tile([P, H * nmsg], F32, tag="agg")
    nc.vector.tensor_reduce(
        out=agg[:],
        in_=svap(prod, 0, [[K * nmsg, H], [1, nmsg], [nmsg, K]]),
        axis=AX.X, op=OP.add)
    ops = work.tile([P, H * nmsg], F32, tag="ops")
    nc.vector.tensor_tensor(
        out=ops[:], in0=agg[:],
        in1=svap(dr, 0, [[1, H], [0, nmsg]]), op=OP.mult)
    out_cb(ops)


def build_fused(Ks):
    nc = bass.Bass(num_devices=NC)
    L1TOT = P * sum(K - 1 for K in Ks)
    RSO, GIO, TOT = _pack_layout(L1TOT)
    pack_d = nc.declare_dram_parameter("pack", [TOT], U16, isOutput=False)
    out_d = nc.declare_dram_parameter("out", [1, G], F32, isOutput=True)

    wb = nc.dram_tensor("wb", [1, SH32], F32)
    wpkg = nc.dram_tensor("wpkg", [1, SH32 * NC], F32, addr_space="Shared")
    xb = nc.dram_tensor("xb", [NLP, IN], U8)
    xg8 = nc.dram_tensor("xg8", [NROWS, IN], U8, addr_space="Shared")
    xg = nc.dram_tensor("xg", [NROWS, IN], F32)
    T1 = nc.dram_tensor("T1", [TROWS, REC1], F32)
    astab = nc.dram_tensor("astab", [NROWS, 4], F32)
    adtab = nc.dram_tensor("adtab", [NROWS, 4], F32)
    t2part = nc.dram_tensor("t2part", [NLP, REC2], F32)
    T2 = nc.dram_tensor("T2", [TROWS, REC2], F32, addr_space="Shared")
    adrow_d = nc.dram_tensor("adrow", [1, NLP], F32)
    pin = nc.dram_tensor("pin", [G, C2], F32)
    pout = nc.dram_tensor("pout", [G, C2], F32, addr_space="Shared")

    with ctile.TileContext(nc) as tc:
        import contextlib
        with contextlib.ExitStack() as ctx:
            const = ctx.enter_context(tc.tile_pool(name="const", bufs=1))
            persist = ctx.enter_context(tc.tile_pool(name="persist", bufs=1))
            work = ctx.enter_context(tc.tile_pool(name="work", bufs=2))
            psum = ctx.enter_context(tc.tile_pool(name="psum", bufs=4, space="PSUM"))
            ppool = ctx.enter_context(tc.tile_pool(name="ppool", bufs=1, space="PSUM"))
            pools = dict(work=work, psum=psum)

            ident = const.tile([P, P], F32)
            make_identity(nc, ident[:])

            # ---- stage 0: all-gather the weight-pack shards ----
            nc.sync.dma_start(
                out=wb[:], in_=vap(pack_d, 0, [[W16, 1], [1, W16]]).bitcast(F32))
            nc.gpsimd.collective_compute(
                "AllGather", OP.bypass, replica_groups=RG,
                ins=[wb[:].opt()], outs=[wpkg[:].opt()])

            a8w_s = const.tile([P, 64], F32)
            nc.sync.dma_start(out=a8w_s[:], in_=vap(wpkg, W_A8W, [[64, P], [1, 64]]))
            w1blk_s = const.tile([64, F1], F32)
            nc.sync.dma_start(out=w1blk_s[:], in_=vap(wpkg, W_W1B, [[128, 64], [1, 128]]))
            b1_s = const.tile([F1, 1], F32)
            nc.sync.dma_start(out=b1_s[:], in_=vap(wpkg, W_B1, [[1, 128], [1, 1]]))
            w2_s = const.tile([F1, C2], F32)
            nc.sync.dma_start(out=w2_s[:], in_=vap(wpkg, W_W2, [[32, 128], [1, 32]]))
            att2_s = const.tile([C2, 2], F32)
            nc.sync.dma_start(out=att2_s[:], in_=vap(wpkg, W_AT2, [[2, 32], [1, 2]]))
            b2bc_s = const.tile([P, C2], F32)
            nc.sync.dma_start(out=b2bc_s[:], in_=vap(wpkg, W_B2, [[0, P], [1, 32]]))
            wh1_s = const.tile([C2, 64], F32)
            nc.sync.dma_start(out=wh1_s[:], in_=vap(wpkg, W_WH1, [[64, 32], [1, 64]]))
            bh1_s = const.tile([64, 1], F32)
            nc.sync.dma_start(out=bh1_s[:], in_=vap(wpkg, W_BH1, [[1, 64], [1, 1]]))
            wh2_s = const.tile([64, 1], F32)
            nc.sync.dma_start(out=wh2_s[:], in_=vap(wpkg, W_WH2, [[1, 64], [1, 1]]))
            bh2_s = const.tile([1, 1], F32)
            nc.sync.dma_start(out=bh2_s[:], in_=vap(wpkg, W_BH2, [[1, 1], [1, 1]]))
            cnt_s = const.tile([G, 1], F32)
            nc.sync.dma_start(out=cnt_s[:], in_=vap(wpkg, W_CNT, [[1, G], [1, 1]]))
            iota_s = const.tile([P, G], F32)
            nc.sync.dma_start(out=iota_s[:], in_=vap(wpkg, W_IOT, [[0, P], [1, G]]))
            rsel16_s = const.tile([P, NT], U16)
            nc.sync.dma_start(out=rsel16_s[:], in_=vap(pack_d, RSO, [[NT, P], [1, NT]]))
            rsel_s = const.tile([P, NT], I32)
            nc.vector.tensor_copy(out=rsel_s[:], in_=rsel16_s[:])
            gid8_s = const.tile([P, GNT], U8)
            nc.sync.dma_start(
                out=gid8_s[:],
                in_=vap(pack_d, GIO, [[GNT // 2, P], [1, GNT // 2]]).bitcast(U8))
            gid_s = const.tile([P, GNT], F32)
            nc.vector.tensor_copy(out=gid_s[:], in_=gid8_s[:])
            xsc_s = const.tile([P, 1], F32)
            nc.sync.dma_start(out=xsc_s[:], in_=vap(wpkg, W_XSC, [[0, P], [1, 1]]))

            # ---- stage 1: all-gather x shards (u8 over the wire) ----
            nc.sync.dma_start(
                out=xb[:], in_=vap(pack_d, XPO, [[IN // 2, NLP], [1, IN // 2]]
                                   ).bitcast(U8))
            nc.gpsimd.collective_compute(
                "AllGather", OP.bypass, replica_groups=RG,
                ins=[xb[:].opt()], outs=[xg8[:].opt()])

            # ---- stage 2: build T1 + per-row logit terms ----
            xt = persist.tile([P, NLQ], F32)
            CH = 512
            for c0 in range(0, NLQ, CH):
                w = min(CH, NLQ - c0)
                ch8 = work.tile([P, CH], U8, tag="ch8")
                nc.sync.dma_start(out=ch8[:, :w],
                                  in_=vap(xg8, c0 * P, [[1, P], [P, w]]))
                chf = work.tile([P, CH], F32, tag="chf")
                nc.vector.tensor_copy(out=chf[:, :w], in_=ch8[:, :w])
                # dequant: (q - 128) * xscale
                nc.vector.scalar_tensor_tensor(
                    out=xt[:, c0:c0 + w], in0=chf[:, :w], scalar=-128.0,
                    in1=svap(xsc_s, 0, [[0, w]]), op0=OP.add, op1=OP.mult)
                nc.sync.dma_start(out=vap(xg, c0 * P, [[1, P], [P, w]]),
                                  in_=xt[:, c0:c0 + w])
                pz = psum.tile([64, CH], F32, tag="ps")
                nc.tensor.matmul(pz[:, :w], lhsT=a8w_s[:], rhs=xt[:, c0:c0 + w],
                                 start=True, stop=True)
                az = work.tile([64, CH], F32, tag="az")
                nc.vector.tensor_copy(out=az[:, :w], in_=pz[:, :w])
                # az partition p=4g+v, col j <-> row 8j+g: astab offset 32j+p.
                nc.sync.dma_start(
                    out=vap(astab, 32 * c0, [[1, 32], [32, w]]), in_=az[0:32, :w])
                nc.sync.dma_start(
                    out=vap(adtab, 32 * c0, [[1, 32], [32, w]]), in_=az[32:64, :w])

            nc.scalar.dma_start(
                out=vap(T1, 0, [[REC1, NROWS], [1, IN]]),
                in_=vap(xg, 0, [[IN, NROWS], [1, IN]]))
            dummy1 = const.tile([1, REC1], F32)
            nc.vector.memset(dummy1[:, 0:IN], 0.0)
            nc.vector.memset(dummy1[:, IN:REC1], -1e30)
            nc.sync.dma_start(out=T1[NROWS:TROWS, :], in_=dummy1[:])
            nc.scalar.dma_start(
                out=vap(T1, IN, [[REC1, NROWS], [1, 4]]),
                in_=vap(astab, 0, [[4, NROWS], [1, 4]]))

            # per-dst a_d for this core's rows, [128, NT*4]
            adS = persist.tile([P, NT * 4], F32)
            for t in range(NT):
                nc.gpsimd.indirect_dma_start(
                    out=adS[:, t * 4:(t + 1) * 4], out_offset=None,
                    in_=adtab[:],
                    in_offset=bass.IndirectOffsetOnAxis(
                        ap=rsel_s[:, t:t + 1], axis=0))

            # ---- stage 3: layer-1 edge phase ----
            h1e = persist.tile([F1, NLP], F32)
            off = 0
            for t in range(NT):
                K = Ks[t]
                idx_dram = vap(pack_d, IXO + off, [[K - 1, P], [1, K - 1]])
                off += P * (K - 1)

                def finish1(ops, t=t):
                    pt = psum.tile([64, P], F32, tag="ps")
                    nc.tensor.transpose(out=pt[:], in_=ops[:], identity=ident[:, :P])
                    opst = work.tile([64, P], F32, tag="opst")
                    nc.vector.tensor_copy(out=opst[:], in_=pt[:])
                    hz = psum.tile([F1, P], F32, tag="ps")
                    nc.tensor.matmul(hz[:], lhsT=w1blk_s[:], rhs=opst[:],
                                     start=True, stop=True)
                    zb = work.tile([F1, P], F32, tag="zb")
                    nc.scalar.activation(zb[:], hz[:], ACT.Identity, bias=b1_s[:])
                    tmin = work.tile([F1, P], F32, tag="tmin")
                    nc.vector.tensor_scalar_min(tmin[:], zb[:], 0.0)
                    te = work.tile([F1, P], F32, tag="te")
                    nc.scalar.activation(te[:], tmin[:], ACT.Exp)
                    trelu = work.tile([F1, P], F32, tag="trelu")
                    nc.vector.tensor_scalar_max(trelu[:], zb[:], 0.0)
                    nc.vector.scalar_tensor_tensor(
                        out=h1e[:, t * P:(t + 1) * P], in0=te[:], scalar=-1.0,
                        in1=trelu[:], op0=OP.add, op1=OP.add)

                edge_softmax_aggregate(
                    nc, tc, pools, idx_dram, T1[:],
                    adS[:, t * 4:(t + 1) * 4], t, K, REC1, IN, H1,
                    rsel_s[:, t:t + 1], finish1)

            # ---- stage 4: layer-2 node phase + T2 all-gather ----
            # h2a rows 0:32 = h2, row 32 = a_s2, row 33 = a_d2
            h2a = persist.tile([C2 + 2, NLP], F32)
            for c0 in range(0, NLP, CH):
                w = min(CH, NLP - c0)
                pz = psum.tile([C2, CH], F32, tag="ps")
                nc.tensor.matmul(pz[:, :w], lhsT=w2_s[:], rhs=h1e[:, c0:c0 + w],
                                 start=True, stop=True)
                nc.vector.tensor_copy(out=h2a[0:C2, c0:c0 + w], in_=pz[:, :w])
                pa = psum.tile([2, CH], F32, tag="ps")
                nc.tensor.matmul(pa[:, :w], lhsT=att2_s[:],
                                 rhs=h2a[0:C2, c0:c0 + w], start=True, stop=True)
                nc.vector.tensor_copy(out=h2a[C2:C2 + 2, c0:c0 + w], in_=pa[:, :w])
            nc.sync.dma_start(out=adrow_d[:], in_=h2a[C2 + 1:C2 + 2, :])
            ad2_s = const.tile([P, NT], F32)
            nc.sync.dma_start(out=ad2_s[:], in_=vap(adrow_d, 0, [[1, P], [P, NT]]))

            for t in range(NT):
                pt = psum.tile([P, C2 + 1], F32, tag="ps")
                nc.tensor.transpose(
                    out=pt[:], in_=h2a[0:C2 + 1, t * P:(t + 1) * P],
                    identity=ident[0:C2 + 1, 0:C2 + 1])
                rec = work.tile([P, REC2], F32, tag="rec")
                nc.vector.tensor_copy(out=rec[:, 0:C2 + 1], in_=pt[:])
                nc.vector.memset(rec[:, C2 + 1:REC2], 0.0)
                nc.sync.dma_start(out=t2part[t * P:(t + 1) * P, :], in_=rec[:])

            nc.gpsimd.collective_compute(
                "AllGather", OP.bypass, replica_groups=RG,
                ins=[t2part[:].opt()], outs=[T2[0:NROWS, :].opt()])
            dummy2 = const.tile([1, REC2], F32)
            nc.vector.memset(dummy2[:, 0:C2], 0.0)
            nc.vector.memset(dummy2[:, C2:REC2], -1e30)
            nc.sync.dma_start(out=T2[NROWS:TROWS, :], in_=dummy2[:])

            # ---- stage 5: layer-2 edge phase + pooling ----
            pooled = ppool.tile([G, C2], F32)
            off = 0
            for t in range(NT):
                K = Ks[t]
                idx_dram = vap(pack_d, IXO + off, [[K - 1, P], [1, K - 1]])
                off += P * (K - 1)

                def finish2(ops, t=t):
                    zb = work.tile([P, C2], F32, tag="zb2")
                    nc.vector.tensor_tensor(out=zb[:], in0=ops[:], in1=b2bc_s[:],
                                            op=OP.add)
                    tmin = work.tile([P, C2], F32, tag="tmin2")
                    nc.vector.tensor_scalar_min(tmin[:], zb[:], 0.0)
                    te = work.tile([P, C2], F32, tag="te2")
                    nc.scalar.activation(te[:], tmin[:], ACT.Exp)
                    trelu = work.tile([P, C2], F32, tag="trelu2")
                    nc.vector.tensor_scalar_max(trelu[:], zb[:], 0.0)
                    hf = work.tile([P, C2], F32, tag="hf")
                    nc.vector.scalar_tensor_tensor(
                        out=hf[:], in0=te[:], scalar=-1.0, in1=trelu[:],
                        op0=OP.add, op1=OP.add)
                    oh = work.tile([P, G], F32, tag="oh")
                    nc.vector.tensor_tensor(
                        out=oh[:], in0=svap(gid_s, t, [[0, G]]),
                        in1=iota_s[:], op=OP.is_equal)
                    nc.tensor.matmul(
                        pooled[:], lhsT=oh[:], rhs=hf[:],
                        start=(t == 0), stop=(t == NT - 1))

                edge_softmax_aggregate(
                    nc, tc, pools, idx_dram, T2[:],
                    ad2_s[:, t:t + 1], t, K, REC2, C2, 1,
                    rsel_s[:, t:t + 1], finish2)

            # ---- stage 6: all-reduce partials + MLP head ----
            po = const.tile([G, C2], F32)
            nc.vector.tensor_copy(out=po[:], in_=pooled[:])
            nc.sync.dma_start(out=pin[:], in_=po[:])
            nc.gpsimd.collective_compute(
                "AllReduce", OP.add, replica_groups=RG,
                ins=[pin[:].opt()], outs=[pout[:].opt()])
            ps = const.tile([G, C2], F32)
            nc.sync.dma_start(out=ps[:], in_=pout[:])

            cm = const.tile([G, 1], F32)
            nc.vector.tensor_scalar_max(cm[:], cnt_s[:], 1.0)
            nc.vector.reciprocal(cm[:], cm[:])
            pooled_s = const.tile([G, C2], F32)
            nc.vector.tensor_scalar_mul(pooled_s[:], ps[:], cm[:])

            pt = psum.tile([C2, G], F32, tag="ps")
            nc.tensor.transpose(out=pt[:], in_=pooled_s[:], identity=ident[:G, :G])
            pooledT = const.tile([C2, G], F32)
            nc.vector.tensor_copy(out=pooledT[:], in_=pt[:])
            z1 = psum.tile([64, G], F32, tag="ps")
            nc.tensor.matmul(z1[:], lhsT=wh1_s[:], rhs=pooledT[:], start=True, stop=True)
            r1 = const.tile([64, G], F32)
            nc.scalar.activation(r1[:], z1[:], ACT.Relu, bias=bh1_s[:])
            z2 = psum.tile([1, G], F32, tag="ps")
            nc.tensor.matmul(z2[:], lhsT=wh2_s[:], rhs=r1[:], start=True, stop=True)
            o = const.tile([1, G], F32)
            nc.scalar.activation(o[:], z2[:], ACT.Identity, bias=bh2_s[:])
            nc.sync.dma_start(out=out_d[:], in_=o[:])
    fix_multiwait(nc)
    return nc


# ---------------------------------------------------------------------------
# cached PJRT runner: build the jitted executable once per (kernel, shapes)
# and reuse it, so repeat calls pay only input transfer + execution.
_RUN_CACHE = {}


def _make_runner(nc, n_cores):
    import jax
    from jax.sharding import Mesh, PartitionSpec
    from jax.experimental.shard_map import shard_map

    bass2jax.install_neuronx_cc_hook()
    partition_name = nc.partition_id_tensor.name if nc.partition_id_tensor else None
    in_names, out_names, out_avals, zero_outs = [], [], [], []
    for alloc in nc.m.functions[0].allocations:
        if not isinstance(alloc, mybir.MemoryLocationSet):
            continue
        name = alloc.memorylocations[0].name
        if alloc.kind == "ExternalInput":
            if name != partition_name:
                in_names.append(name)
        elif alloc.kind == "ExternalOutput":
            out_names.append(name)
            shape = tuple(alloc.tensor_shape)
            dtype = mybir.dt.np(alloc.dtype)
            out_avals.append(jax.core.ShapedArray(shape, dtype))
            zero_outs.append(np.zeros(shape, dtype))
    n_params = len(in_names)
    n_outs = len(out_avals)
    all_in_names = list(in_names) + out_names + (
        [partition_name] if partition_name else [])

    def _body(*args):
        operands = list(args)
        if partition_name is not None:
            operands.append(bass2jax.partition_id_tensor())
        outs = bass2jax._bass_exec_p.bind(
            *operands, out_avals=tuple(out_avals), in_names=tuple(all_in_names),
            out_names=tuple(out_names), lowering_input_output_aliases=(),
            sim_require_finite=True, sim_require_nnan=True, nc=nc)
        return tuple(outs)

    donate = tuple(range(n_params, n_params + n_outs))
    devices = jax.devices()[:n_cores]
    assert len(devices) == n_cores
    mesh = Mesh(np.asarray(devices), ("core",))
    in_specs = (PartitionSpec("core"),) * (n_params + n_outs)
    out_specs = (PartitionSpec("core"),) * len(out_names)
    sharded = jax.jit(shard_map(_body, mesh=mesh, in_specs=in_specs,
                                out_specs=out_specs, check_rep=False),
                      donate_argnums=donate, keep_unused=True)

    def run(in_maps):
        per_core = [[np.asarray(m[name]) for name in in_names] for m in in_maps]
        concat_in = [np.concatenate([per_core[c][i] for c in range(n_cores)], axis=0)
                     for i in range(n_params)]
        concat_zeros = [np.zeros((n_cores * z.shape[0], *z.shape[1:]), z.dtype)
                        for z in zero_outs]
        out_arrs = sharded(*concat_in, *concat_zeros)
        return [{name: np.asarray(out_arrs[i]).reshape(n_cores, *out_avals[i].shape)[c]
                 for i, name in enumerate(out_names)}
                for c in range(n_cores)]
    return run


def _get_runner(key, build_fn):
    if key not in _RUN_CACHE:
        _RUN_CACHE[key] = _make_runner(build_fn(), NC)
    return _RUN_CACHE[key]


def _null_nc():
    nc = bass.Bass()
    x = nc.declare_dram_parameter("x", [P, 64], F32, isOutput=False)
    y = nc.declare_dram_parameter("y", [P, 64], F32, isOutput=True)
    with ctile.TileContext(nc) as tc:
        with tc.tile_pool(name="sbuf", bufs=1) as pool:
            t = pool.tile([P, 64], F32)
            nc.sync.dma_start(out=t[:], in_=x[:])
            nc.sync.dma_start(out=y[:], in_=t[:])
    fix_multiwait(nc)
    return nc


# ---------------------------------------------------------------------------
def _make_inmaps(prep, wpack):
    return [dict(pack=pk) for pk in _build_packs(prep, wpack)]


def kernel(x, edge_index, batch, W1, att_src1, att_dst1, b1,
           W2, att_src2, att_dst2, b2, Wh1, bh1, Wh2, bh2):
    prep = host_prep(x, edge_index, batch)
    wpack = fold_weights(W1, att_src1, att_dst1, b1, W2, att_src2, att_dst2,
                         b2, Wh1, bh1, Wh2, bh2, prep["cnt"], prep["xscale"])
    run = _get_runner(("fused", tuple(prep["Ks"])),
                      lambda: build_fused(prep["Ks"]))
    res = run(_make_inmaps(prep, wpack))
    return res[0]["out"].reshape(G, 1).astype(np.float32)


def _wall_min(fn, n=5):
    import time
    best = 1e9
    for _ in range(n):
        t0 = time.perf_counter()
        fn()
        best = min(best, time.perf_counter() - t0)
    return best


def timed_run(inputs):
    """Estimate on-device exec ns: warm per-call wall minus null-kernel wall.

    The axon PJRT path exposes no NTFF profiling, so this is an upper-bound
    estimate: warm per-call wall (input transfer + execution + output fetch)
    minus the warm wall of a trivial kernel (same dispatch/tunnel overhead),
    floored at 0.
    """
    prep = host_prep(inputs["x"], inputs["edge_index"], inputs["batch"])
    wpack = fold_weights(inputs["W1"], inputs["att_src1"], inputs["att_dst1"],
                         inputs["b1"], inputs["W2"], inputs["att_src2"],
                         inputs["att_dst2"], inputs["b2"], inputs["Wh1"],
                         inputs["bh1"], inputs["Wh2"], inputs["bh2"],
                         prep["cnt"], prep["xscale"])
    in_maps = _make_inmaps(prep, wpack)

    run0 = _get_runner(("null",), _null_nc)
    im0 = [dict(x=np.zeros((P, 64), np.float32)) for _ in range(NC)]
    run0(im0)
    t0 = _wall_min(lambda: run0(im0), n=5)

    run = _get_runner(("fused", tuple(prep["Ks"])),
                      lambda: build_fused(prep["Ks"]))
    run(in_maps)
    t1 = _wall_min(lambda: run(in_maps), n=5)

    d1 = max(t1 - t0, 0.0)
    print(f"null wall {t0*1e3:.1f} ms; fused launch {t1*1e3:.1f} ms")
    print(f"fused exec est {d1*1e6:.0f} us")
    return d1 * 1e9
